# revision 3
# baseline (speedup 1.0000x reference)
"""Trainium2 Bass kernel for nn_Attention_8744553414813.

Reference computation (B=4, C=512, H=W=64, HW=4096):
    Q = conv1x1(mean_norm(content), Wq, bq)   # [B, C, HW]
    K = conv1x1(mean_norm(style),   Wk, bk)
    V = conv1x1(style,              Wv, bv)
    A = softmax(Q^T K, axis=-1)               # [B, HWc, HWs]
    out = V @ A^T                             # [B, C, HW]

Sharding: 8 cores = 4 batches x 2 content-pixel halves (data parallel; the
small 1x1-conv weights are replicated). Each core computes out^T for its
2048 query pixels; the host transposes and reassembles.

Host-side folding: the per-channel mean/std of content/style are folded
into the conv weights (W' = W*diag(1/std), b' = b - W'^T mean), so the
device consumes raw pixels and pre-folded weights; style is streamed ONCE
(K and V projected from the same chunks), content once (this core's half).

Numerics: fp16 and tf32 have the same 11-bit effective mantissa, so the
projections run entirely in fp16 (halves input DMA bytes + LDWEIGHTS time)
at identical accuracy; the error-critical Q/K score values stay f32r.
The K bias is dropped: it only adds a per-query constant to scores, which
softmax is invariant to.

Per-core device program:
 - warmup identity transposes fill the DMA-wait window and start the PE
   DVFS ramp early
 - V^T and K projected per style chunk in fp16; kt stored f32r (no bias),
   V^T fp16; Q projected in fp16, qc stored f32r with b'q via DVE
 - scores in f32r; flash-style online softmax over four 1024-col PSUM
   chunks; exp on the scalar engine with per-partition bias and fused
   row-sum (accum_out)
 - A in fp16; A^T via PE transposes, 8 per fp16 PSUM bank
 - AV matmul accumulates out^T [q, c]; 1/d and bv applied in the epilogue
 - software pipelining: each q-tile's transpose+AV is emitted after the
   NEXT tile's score matmuls so the PE fills the softmax latency
"""
import numpy as np

import concourse.bacc as bacc
import concourse.bass as bass
import concourse.mybir as mybir
import concourse.tile as tile
from concourse.bass_utils import run_bass_kernel_spmd
from concourse.masks import make_identity

F32 = mybir.dt.float32
F32R = mybir.dt.float32r
F16 = mybir.dt.float16
AF = mybir.ActivationFunctionType
AX = mybir.AxisListType
OP = mybir.AluOpType

B, C, H, W = 4, 512, 64, 64
HW = H * W                  # 4096 (style/key pixels per core)
QN = HW // 2                # 2048 query pixels per core
CS = C // 128               # 4 channel sub-tiles
EPS = 1e-5
KCHUNK = 1024               # scores psum chunk width (2 banks)
NKC = HW // KCHUNK          # 4 online-softmax chunks
PIX = 512                   # projection pixel chunk
NSC = HW // PIX             # 8 style chunks
NCC = QN // PIX             # 4 content chunks per core
NWARM = 24                  # PE warmup transposes


def build_nc():
    nc = bacc.Bacc(trn_type="TRN2")
    # chunk-major packed pixels: [ci, chunk, sub, px], fp16
    xs = nc.dram_tensor("xs_p", [128, NSC, CS, PIX], F16, kind="ExternalInput")
    xc = nc.dram_tensor("xc_p", [128, NCC, CS, PIX], F16, kind="ExternalInput")
    # folded weights packed [ci, 3(q,k,v), sub, cout], fp16
    wp = nc.dram_tensor("w_p", [128, 3, CS, C], F16, kind="ExternalInput")
    # biases packed [p, 0:4]=b'q, [4:516]=bv broadcast
    bp = nc.dram_tensor("b_p", [128, CS + C], F32, kind="ExternalInput")
    out = nc.dram_tensor("out_t", [QN, C], F32, kind="ExternalOutput")  # out^T for this core

    with tile.TileContext(nc) as tc:
        with tc.tile_pool(name="sb", bufs=1) as sb, \
             tc.tile_pool(name="cst", bufs=1) as cst, \
             tc.tile_pool(name="xsp", bufs=3) as xsp, \
             tc.tile_pool(name="xcp", bufs=2) as xcp, \
             tc.tile_pool(name="qc", bufs=1) as qcp, \
             tc.tile_pool(name="ab", bufs=2) as abp, \
             tc.tile_pool(name="atb", bufs=1) as atp, \
             tc.tile_pool(name="ob", bufs=2) as obp, \
             tc.tile_pool(name="sm", bufs=2) as smp, \
             tc.tile_pool(name="psS", bufs=2, space="PSUM") as psS, \
             tc.tile_pool(name="psT", bufs=2, space="PSUM") as psT, \
             tc.tile_pool(name="psM", bufs=2, space="PSUM") as psM:

            # ---------- constants (weights first: they gate the first matmul) ----------
            wsb = cst.tile([128, 3, CS, C], F16)
            nc.sync.dma_start(wsb[:], wp[:])
            bsb = cst.tile([128, CS + C], F32)
            nc.sync.dma_start(bsb[:], bp[:])
            ident = cst.tile([128, 128], F16)
            make_identity(nc, ident)

            # PE warmup while the first DMAs land: ramps the clock
            for i in range(NWARM):
                wt = psT.tile([128, 1024], F16, tag="tp")
                for j in range(8):
                    nc.tensor.transpose(wt[:, j * 128:(j + 1) * 128], ident[:], ident[:])

            wq_r = wsb[:, 0]
            wk_r = wsb[:, 1]
            wv_r = wsb[:, 2]
            bq_t = bsb[:, 0:CS]
            bv_t = bsb[:, CS:]

            vt = sb.tile([128, HW // 128, C], F16)           # V^T [k, cout], 32 KB/p
            kt = sb.tile([128, CS, HW], F32R)                # K [cout, k], 64 KB/p

            xqts = [None] * NCC

            def issue_xq(t):
                xqt = xcp.tile([128, CS, PIX], F16, tag="xcp")
                nc.scalar.dma_start(xqt[:], xc[:, t])
                xqts[t] = xqt

            # ---------- single style pass: V^T and K projections per chunk ----------
            for t in range(NSC):
                xst = xsp.tile([128, CS, PIX], F16, tag="xs")
                nc.sync.dma_start(xst[:], xs[:, t])
                for ks in range(PIX // 128):
                    psv = psM.tile([128, C], F32, tag="mm512")
                    for sub in range(CS):
                        nc.tensor.matmul(psv[:], xst[:, sub, ks * 128:(ks + 1) * 128],
                                         wv_r[:, sub, :], start=(sub == 0), stop=(sub == CS - 1))
                    nc.scalar.copy(vt[:, t * (PIX // 128) + ks, :], psv[:])
                for co in range(CS):
                    psk = psM.tile([128, PIX], F32, tag="mm512")
                    for ci in range(CS):
                        nc.tensor.matmul(psk[:], wk_r[:, ci, co * 128:(co + 1) * 128],
                                         xst[:, ci, :], start=(ci == 0), stop=(ci == CS - 1))
                    nc.scalar.copy(kt[:, co, t * PIX:(t + 1) * PIX], psk[:])
                if t >= NSC - 2:
                    issue_xq(t - (NSC - 2))   # prefetch content chunks 0,1 late

            # ---------- Q projection + attention (software pipelined) ----------
            pend = None   # (at, rd, q0) of the previous q-tile

            def flush(p):
                at_p, rd_p, q0_p = p
                att = atp.tile([128, HW // 128, 128], F16, tag="AT")
                for g in range(HW // 128 // 8):
                    tp = psT.tile([128, 1024], F16, tag="tp")
                    for i in range(8):
                        kb = g * 8 + i
                        nc.tensor.transpose(tp[:, i * 128:(i + 1) * 128],
                                            at_p[:, kb * 128:(kb + 1) * 128], ident[:])
                    nc.scalar.copy(att[:, g * 8:(g + 1) * 8, :], tp[:])
                av = psM.tile([128, C], F32, tag="mm512")
                for kb in range(HW // 128):
                    nc.tensor.matmul(av[:], att[:, kb, :], vt[:, kb, :],
                                     start=(kb == 0), stop=(kb == HW // 128 - 1))
                ot = obp.tile([128, C], F32, tag="ot")
                nc.vector.tensor_scalar_mul(ot[:], av[:], rd_p[:])
                nc.vector.tensor_tensor(ot[:], ot[:], bv_t[:], OP.add)
                nc.sync.dma_start(out[q0_p:q0_p + 128, :], ot[:])

            for t in range(NCC):
                if t + 2 < NCC:
                    issue_xq(t + 2)
                xqt = xqts[t]
                qc = qcp.tile([128, CS, PIX], F32R, tag="qc")
                for co in range(CS):
                    psq = psM.tile([128, PIX], F32, tag="mm512")
                    for ci in range(CS):
                        nc.tensor.matmul(psq[:], wq_r[:, ci, co * 128:(co + 1) * 128],
                                         xqt[:, ci, :], start=(ci == 0), stop=(ci == CS - 1))
                    nc.vector.tensor_scalar_add(qc[:, co, :], psq[:], bq_t[:, co:co + 1])

                for j in range(PIX // 128):          # q-tile of 128 queries
                    at = abp.tile([128, HW], F16, tag="A")
                    mruns = smp.tile([128, NKC], F32, tag="mruns")
                    negs = smp.tile([128, NKC], F32, tag="negs")
                    dvec = smp.tile([128, NKC], F32, tag="dvec")
                    for kc in range(NKC):
                        sps = psS.tile([128, KCHUNK], F32, tag="s")
                        for kb in range(KCHUNK // PIX):
                            koff = kc * KCHUNK + kb * PIX
                            for sub in range(CS):
                                nc.tensor.matmul(sps[:, kb * PIX:(kb + 1) * PIX],
                                                 qc[:, sub, j * 128:(j + 1) * 128],
                                                 kt[:, sub, koff:koff + PIX],
                                                 start=(sub == 0), stop=(sub == CS - 1))
                        if kc == 0:
                            nc.vector.reduce_max(mruns[:, 0:1], sps[:], axis=AX.X)
                        else:
                            mx = smp.tile([128, 1], F32, tag="mx")
                            nc.vector.reduce_max(mx[:], sps[:], axis=AX.X)
                            nc.vector.tensor_tensor(mruns[:, kc:kc + 1], mruns[:, kc - 1:kc],
                                                    mx[:], OP.max)
                        nc.vector.tensor_scalar_mul(negs[:, kc:kc + 1], mruns[:, kc:kc + 1], -1.0)
                        nc.scalar.activation(at[:, kc * KCHUNK:(kc + 1) * KCHUNK], sps[:],
                                             AF.Exp, bias=negs[:, kc:kc + 1], scale=1.0,
                                             accum_out=dvec[:, kc:kc + 1])
                    fac = smp.tile([128, NKC], F32, tag="fac")
                    nc.scalar.activation(fac[:], mruns[:], AF.Exp,
                                         bias=negs[:, NKC - 1:NKC], scale=1.0)
                    dsc = smp.tile([128, NKC], F32, tag="dsc")
                    nc.vector.tensor_tensor(dsc[:], dvec[:], fac[:], OP.mult)
                    dtot = smp.tile([128, 1], F32, tag="dtot")
                    nc.vector.reduce_sum(dtot[:], dsc[:], axis=AX.X)
                    rd = smp.tile([128, 1], F32, tag="rd")
                    nc.vector.reciprocal(rd[:], dtot[:])
                    for kc in range(NKC - 1):
                        nc.vector.tensor_scalar_mul(at[:, kc * KCHUNK:(kc + 1) * KCHUNK],
                                                    at[:, kc * KCHUNK:(kc + 1) * KCHUNK],
                                                    fac[:, kc:kc + 1])
                    if pend is not None:
                        flush(pend)
                    pend = (at, rd, (t * PIX // 128 + j) * 128)
            flush(pend)

    nc.compile()
    return nc


_NC = None
_last_in_maps = None


def _get_nc():
    global _NC
    if _NC is None:
        _NC = build_nc()
    return _NC


def _fold(feat, Wt, b):
    """Fold channel mean/std normalization into W^T [cin,cout] and b [cout]."""
    x = feat.reshape(C, HW).astype(np.float64)
    mean = x.mean(axis=1)
    var = ((x - mean[:, None]) ** 2).sum(axis=1) / (HW - 1)
    std = np.sqrt(var + EPS)
    Wp = (Wt / std[:, None].astype(np.float32)).astype(np.float16)
    bp = (b.astype(np.float64) - Wp.astype(np.float64).T @ mean).astype(np.float32)
    return Wp, bp


def _pack_w(Wt):
    """[cin, cout] -> [ci, sub, cout] with cin = sub*128 + ci."""
    return np.ascontiguousarray(Wt.reshape(CS, 128, C).transpose(1, 0, 2))


def _pack_x(x, nchunk):
    """[C, n*512] f32 -> chunk-major fp16 [ci, chunk, sub, px]."""
    return np.ascontiguousarray(
        x.astype(np.float16).reshape(CS, 128, nchunk, PIX).transpose(1, 2, 0, 3))


def kernel(content_feat, style_feat, Wq, bq, Wk, bk, Wv, bv):
    content = np.asarray(content_feat, dtype=np.float32).reshape(B, C, HW)
    style = np.asarray(style_feat, dtype=np.float32).reshape(B, C, HW)
    Wq = np.asarray(Wq, dtype=np.float32)
    Wk = np.asarray(Wk, dtype=np.float32)
    Wv = np.asarray(Wv, dtype=np.float32)
    bq = np.asarray(bq, dtype=np.float32)
    bk = np.asarray(bk, dtype=np.float32)
    bv = np.asarray(bv, dtype=np.float32)

    in_maps = []
    per_batch = {}
    for b in range(B):
        wq_p, bq_p = _fold(content[b], Wq.T.copy(), bq)
        wk_p, _ = _fold(style[b], Wk.T.copy(), bk)   # b'k is softmax-invariant
        wv_p = Wv.T.astype(np.float16)
        w_p = np.ascontiguousarray(
            np.stack([_pack_w(wq_p), _pack_w(wk_p), _pack_w(wv_p)], axis=1))
        b_p = np.empty((128, CS + C), np.float32)
        b_p[:, 0:CS] = bq_p.reshape(CS, 128).T
        b_p[:, CS:] = bv[None, :]
        per_batch[b] = (w_p, b_p, _pack_x(style[b], NSC))

    for core in range(8):
        b = core // 2
        half = core % 2
        w_p, b_p, xs_p = per_batch[b]
        xc_half = content[b][:, half * QN:(half + 1) * QN]
        in_maps.append({
            "xs_p": xs_p,
            "xc_p": _pack_x(xc_half, NCC),
            "w_p": w_p,
            "b_p": b_p,
        })

    global _last_in_maps
    _last_in_maps = in_maps
    nc = _get_nc()
    res = run_bass_kernel_spmd(nc, in_maps, core_ids=list(range(8)))

    outf = np.empty((B, C, HW), dtype=np.float32)
    for core in range(8):
        b = core // 2
        half = core % 2
        ot = np.asarray(res.results[core]["out_t"])  # [QN, C]
        outf[b, :, half * QN:(half + 1) * QN] = ot.T
    return outf.reshape(B, C, H, W)


if __name__ == "__main__":
    rng = np.random.default_rng(0)
    inputs = {
        "content_feat": rng.standard_normal((B, C, H, W), dtype=np.float32),
        "style_feat": rng.standard_normal((B, C, H, W), dtype=np.float32),
        "Wq": rng.standard_normal((C, C), dtype=np.float32) * 0.05,
        "bq": rng.random(C, dtype=np.float32),
        "Wk": rng.standard_normal((C, C), dtype=np.float32) * 0.05,
        "bk": rng.random(C, dtype=np.float32),
        "Wv": rng.standard_normal((C, C), dtype=np.float32) * 0.05,
        "bv": rng.random(C, dtype=np.float32),
    }
    out = kernel(**inputs)
    print("kernel output:", out.shape, out.dtype, float(np.abs(out).max()))


# revision 4
# speedup vs baseline: 1.3531x; 1.3531x over previous
"""Trainium2 Bass kernel for nn_Attention_8744553414813.

Reference computation (B=4, C=512, H=W=64, HW=4096):
    Q = conv1x1(mean_norm(content), Wq, bq)   # [B, C, HW]
    K = conv1x1(mean_norm(style),   Wk, bk)
    V = conv1x1(style,              Wv, bv)
    A = softmax(Q^T K, axis=-1)               # [B, HWc, HWs]
    out = V @ A^T                             # [B, C, HW]

Sharding: 8 cores = 4 batches x 2 content-pixel halves (data parallel; the
small 1x1-conv weights are replicated). Each core computes out^T for its
2048 query pixels; the host transposes and reassembles.

Host-side folding: the per-channel mean/std of content/style are folded
into the conv weights (W' = W*diag(1/std), b' = b - W'^T mean), so the
device consumes raw pixels and pre-folded weights; style is streamed ONCE
(K and V projected from the same chunks), content once (this core's half).

Numerics: fp16 and tf32 have the same 11-bit effective mantissa, so the
projections run entirely in fp16 (halves input DMA bytes + LDWEIGHTS time)
at identical accuracy; the error-critical Q/K score values stay f32r.
The K bias is dropped: it only adds a per-query constant to scores, which
softmax is invariant to.

Per-core device program:
 - warmup identity transposes fill the DMA-wait window and start the PE
   DVFS ramp early
 - V^T and K projected per style chunk in fp16; kt stored f32r (no bias),
   V^T fp16; Q projected in fp16, qc stored f32r with b'q via DVE
 - scores in f32r; flash-style online softmax over four 1024-col PSUM
   chunks; exp on the scalar engine with per-partition bias and fused
   row-sum (accum_out)
 - A in fp16; A^T via PE transposes, 8 per fp16 PSUM bank
 - AV matmul accumulates out^T [q, c]; 1/d and bv applied in the epilogue
 - software pipelining: each q-tile's transpose+AV is emitted after the
   NEXT tile's score matmuls so the PE fills the softmax latency
"""
import numpy as np

import concourse.bacc as bacc
import concourse.bass as bass
import concourse.mybir as mybir
import concourse.tile as tile
from concourse.bass_utils import run_bass_kernel_spmd
from concourse.masks import make_identity

F32 = mybir.dt.float32
F32R = mybir.dt.float32r
F16 = mybir.dt.float16
AF = mybir.ActivationFunctionType
AX = mybir.AxisListType
OP = mybir.AluOpType

B, C, H, W = 4, 512, 64, 64
HW = H * W                  # 4096 (style/key pixels per core)
QN = HW // 2                # 2048 query pixels per core
CS = C // 128               # 4 channel sub-tiles
EPS = 1e-5
KCHUNK = 1024               # scores psum chunk width (2 banks)
NKC = HW // KCHUNK          # 4 online-softmax chunks
PIX = 512                   # projection pixel chunk
NSC = HW // PIX             # 8 style chunks
NCC = QN // PIX             # 4 content chunks per core
NWARM = 8                   # PE warmup transposes


def build_nc():
    nc = bacc.Bacc(trn_type="TRN2")
    # chunk-major packed pixels: [ci, chunk, sub, px], fp16
    xs = nc.dram_tensor("xs_p", [128, NSC, CS, PIX], F16, kind="ExternalInput")
    xc = nc.dram_tensor("xc_p", [128, NCC, CS, PIX], F16, kind="ExternalInput")
    # folded weights packed [ci, 3(q,k,v), sub, cout], fp16
    wp = nc.dram_tensor("w_p", [128, 3, CS, C], F16, kind="ExternalInput")
    # biases packed [p, 0:4]=b'q, [4:516]=bv broadcast
    bp = nc.dram_tensor("b_p", [128, CS + C], F32, kind="ExternalInput")
    out = nc.dram_tensor("out_t", [QN, C], F32, kind="ExternalOutput")  # out^T for this core

    with tile.TileContext(nc) as tc:
        with tc.tile_pool(name="sb", bufs=1) as sb, \
             tc.tile_pool(name="cst", bufs=1) as cst, \
             tc.tile_pool(name="xsp", bufs=3) as xsp, \
             tc.tile_pool(name="xcp", bufs=2) as xcp, \
             tc.tile_pool(name="qc", bufs=1) as qcp, \
             tc.tile_pool(name="ab", bufs=2) as abp, \
             tc.tile_pool(name="atb", bufs=1) as atp, \
             tc.tile_pool(name="ob", bufs=2) as obp, \
             tc.tile_pool(name="sm", bufs=2) as smp, \
             tc.tile_pool(name="psS", bufs=2, space="PSUM") as psS, \
             tc.tile_pool(name="psT", bufs=2, space="PSUM") as psT, \
             tc.tile_pool(name="psM", bufs=2, space="PSUM") as psM:

            # ---------- constants (weights first: they gate the first matmul) ----------
            wsb = cst.tile([128, 3, CS, C], F16)
            nc.sync.dma_start(wsb[:], wp[:])
            bsb = cst.tile([128, CS + C], F32)
            nc.sync.dma_start(bsb[:], bp[:])
            ident = cst.tile([128, 128], F16)
            make_identity(nc, ident)

            # PE warmup while the first DMAs land: ramps the clock
            for i in range(NWARM):
                wt = psT.tile([128, 1024], F16, tag="tp")
                for j in range(8):
                    nc.tensor.transpose(wt[:, j * 128:(j + 1) * 128], ident[:], ident[:])

            wq_r = wsb[:, 0]
            wk_r = wsb[:, 1]
            wv_r = wsb[:, 2]
            bq_t = bsb[:, 0:CS]
            bv_t = bsb[:, CS:]

            vt = sb.tile([128, HW // 128, C], F16)           # V^T [k, cout], 32 KB/p
            kt = sb.tile([128, CS, HW], F32R)                # K [cout, k], 64 KB/p

            xqts = [None] * NCC

            def issue_xq(t):
                xqt = xcp.tile([128, CS, PIX], F16, tag="xcp")
                nc.scalar.dma_start(xqt[:], xc[:, t])
                xqts[t] = xqt

            # ---------- single style pass: V^T and K projections per chunk ----------
            for t in range(NSC):
                xst = xsp.tile([128, CS, PIX], F16, tag="xs")
                nc.sync.dma_start(xst[:], xs[:, t])
                for ks in range(PIX // 128):
                    psv = psM.tile([128, C], F32, tag="mm512")
                    for sub in range(CS):
                        nc.tensor.matmul(psv[:], xst[:, sub, ks * 128:(ks + 1) * 128],
                                         wv_r[:, sub, :], start=(sub == 0), stop=(sub == CS - 1))
                    nc.scalar.copy(vt[:, t * (PIX // 128) + ks, :], psv[:])
                for co in range(CS):
                    psk = psM.tile([128, PIX], F32, tag="mm512")
                    for ci in range(CS):
                        nc.tensor.matmul(psk[:], wk_r[:, ci, co * 128:(co + 1) * 128],
                                         xst[:, ci, :], start=(ci == 0), stop=(ci == CS - 1))
                    nc.scalar.copy(kt[:, co, t * PIX:(t + 1) * PIX], psk[:])
                if t >= NSC - 2:
                    issue_xq(t - (NSC - 2))   # prefetch content chunks 0,1 late

            # ---------- Q projection + attention (software pipelined) ----------
            pend = None   # (at, rd, q0) of the previous q-tile

            def flush(p):
                at_p, rd_p, q0_p = p
                att = atp.tile([128, HW // 128, 128], F16, tag="AT")
                for g in range(HW // 128 // 8):
                    tp = psT.tile([128, 1024], F16, tag="tp")
                    for i in range(8):
                        kb = g * 8 + i
                        nc.tensor.transpose(tp[:, i * 128:(i + 1) * 128],
                                            at_p[:, kb * 128:(kb + 1) * 128], ident[:])
                    nc.scalar.copy(att[:, g * 8:(g + 1) * 8, :], tp[:])
                av = psM.tile([128, C], F32, tag="mm512")
                for kb in range(HW // 128):
                    nc.tensor.matmul(av[:], att[:, kb, :], vt[:, kb, :],
                                     start=(kb == 0), stop=(kb == HW // 128 - 1))
                ot = obp.tile([128, C], F32, tag="ot")
                nc.vector.tensor_scalar_mul(ot[:], av[:], rd_p[:])
                nc.vector.tensor_tensor(ot[:], ot[:], bv_t[:], OP.add)
                nc.sync.dma_start(out[q0_p:q0_p + 128, :], ot[:])

            for t in range(NCC):
                if t + 2 < NCC:
                    issue_xq(t + 2)
                xqt = xqts[t]
                qc = qcp.tile([128, CS, PIX], F32R, tag="qc")
                for co in range(CS):
                    psq = psM.tile([128, PIX], F32, tag="mm512")
                    for ci in range(CS):
                        nc.tensor.matmul(psq[:], wq_r[:, ci, co * 128:(co + 1) * 128],
                                         xqt[:, ci, :], start=(ci == 0), stop=(ci == CS - 1))
                    nc.vector.tensor_scalar_add(qc[:, co, :], psq[:], bq_t[:, co:co + 1])

                for j in range(PIX // 128):          # q-tile of 128 queries
                    at = abp.tile([128, HW], F16, tag="A")
                    mruns = smp.tile([128, NKC], F32, tag="mruns")
                    negs = smp.tile([128, NKC], F32, tag="negs")
                    dvec = smp.tile([128, NKC], F32, tag="dvec")
                    for kc in range(NKC):
                        sps = psS.tile([128, KCHUNK], F32, tag="s")
                        for kb in range(KCHUNK // PIX):
                            koff = kc * KCHUNK + kb * PIX
                            for sub in range(CS):
                                nc.tensor.matmul(sps[:, kb * PIX:(kb + 1) * PIX],
                                                 qc[:, sub, j * 128:(j + 1) * 128],
                                                 kt[:, sub, koff:koff + PIX],
                                                 start=(sub == 0), stop=(sub == CS - 1))
                        if kc == 0:
                            nc.vector.reduce_max(mruns[:, 0:1], sps[:], axis=AX.X)
                        else:
                            mx = smp.tile([128, 1], F32, tag="mx")
                            nc.vector.reduce_max(mx[:], sps[:], axis=AX.X)
                            nc.vector.tensor_tensor(mruns[:, kc:kc + 1], mruns[:, kc - 1:kc],
                                                    mx[:], OP.max)
                        nc.vector.tensor_scalar_mul(negs[:, kc:kc + 1], mruns[:, kc:kc + 1], -1.0)
                        nc.scalar.activation(at[:, kc * KCHUNK:(kc + 1) * KCHUNK], sps[:],
                                             AF.Exp, bias=negs[:, kc:kc + 1], scale=1.0,
                                             accum_out=dvec[:, kc:kc + 1])
                    fac = smp.tile([128, NKC], F32, tag="fac")
                    nc.scalar.activation(fac[:], mruns[:], AF.Exp,
                                         bias=negs[:, NKC - 1:NKC], scale=1.0)
                    dsc = smp.tile([128, NKC], F32, tag="dsc")
                    nc.vector.tensor_tensor(dsc[:], dvec[:], fac[:], OP.mult)
                    dtot = smp.tile([128, 1], F32, tag="dtot")
                    nc.vector.reduce_sum(dtot[:], dsc[:], axis=AX.X)
                    rd = smp.tile([128, 1], F32, tag="rd")
                    nc.vector.reciprocal(rd[:], dtot[:])
                    for kc in range(NKC - 1):
                        nc.vector.tensor_scalar_mul(at[:, kc * KCHUNK:(kc + 1) * KCHUNK],
                                                    at[:, kc * KCHUNK:(kc + 1) * KCHUNK],
                                                    fac[:, kc:kc + 1])
                    if pend is not None:
                        flush(pend)
                    pend = (at, rd, (t * PIX // 128 + j) * 128)
            flush(pend)

    nc.compile()
    return nc


_NC = None
_last_in_maps = None


def _get_nc():
    global _NC
    if _NC is None:
        _NC = build_nc()
    return _NC


def _fold(feat, Wt, b):
    """Fold channel mean/std normalization into W^T [cin,cout] and b [cout]."""
    x = feat.reshape(C, HW).astype(np.float64)
    mean = x.mean(axis=1)
    var = ((x - mean[:, None]) ** 2).sum(axis=1) / (HW - 1)
    std = np.sqrt(var + EPS)
    Wp = (Wt / std[:, None].astype(np.float32)).astype(np.float16)
    bp = (b.astype(np.float64) - Wp.astype(np.float64).T @ mean).astype(np.float32)
    return Wp, bp


def _pack_w(Wt):
    """[cin, cout] -> [ci, sub, cout] with cin = sub*128 + ci."""
    return np.ascontiguousarray(Wt.reshape(CS, 128, C).transpose(1, 0, 2))


def _pack_x(x, nchunk):
    """[C, n*512] f32 -> chunk-major fp16 [ci, chunk, sub, px]."""
    return np.ascontiguousarray(
        x.astype(np.float16).reshape(CS, 128, nchunk, PIX).transpose(1, 2, 0, 3))


def kernel(content_feat, style_feat, Wq, bq, Wk, bk, Wv, bv):
    content = np.asarray(content_feat, dtype=np.float32).reshape(B, C, HW)
    style = np.asarray(style_feat, dtype=np.float32).reshape(B, C, HW)
    Wq = np.asarray(Wq, dtype=np.float32)
    Wk = np.asarray(Wk, dtype=np.float32)
    Wv = np.asarray(Wv, dtype=np.float32)
    bq = np.asarray(bq, dtype=np.float32)
    bk = np.asarray(bk, dtype=np.float32)
    bv = np.asarray(bv, dtype=np.float32)

    in_maps = []
    per_batch = {}
    for b in range(B):
        wq_p, bq_p = _fold(content[b], Wq.T.copy(), bq)
        wk_p, _ = _fold(style[b], Wk.T.copy(), bk)   # b'k is softmax-invariant
        wv_p = Wv.T.astype(np.float16)
        w_p = np.ascontiguousarray(
            np.stack([_pack_w(wq_p), _pack_w(wk_p), _pack_w(wv_p)], axis=1))
        b_p = np.empty((128, CS + C), np.float32)
        b_p[:, 0:CS] = bq_p.reshape(CS, 128).T
        b_p[:, CS:] = bv[None, :]
        per_batch[b] = (w_p, b_p, _pack_x(style[b], NSC))

    for core in range(8):
        b = core // 2
        half = core % 2
        w_p, b_p, xs_p = per_batch[b]
        xc_half = content[b][:, half * QN:(half + 1) * QN]
        in_maps.append({
            "xs_p": xs_p,
            "xc_p": _pack_x(xc_half, NCC),
            "w_p": w_p,
            "b_p": b_p,
        })

    global _last_in_maps
    _last_in_maps = in_maps
    nc = _get_nc()
    res = run_bass_kernel_spmd(nc, in_maps, core_ids=list(range(8)))

    outf = np.empty((B, C, HW), dtype=np.float32)
    for core in range(8):
        b = core // 2
        half = core % 2
        ot = np.asarray(res.results[core]["out_t"])  # [QN, C]
        outf[b, :, half * QN:(half + 1) * QN] = ot.T
    return outf.reshape(B, C, H, W)


if __name__ == "__main__":
    rng = np.random.default_rng(0)
    inputs = {
        "content_feat": rng.standard_normal((B, C, H, W), dtype=np.float32),
        "style_feat": rng.standard_normal((B, C, H, W), dtype=np.float32),
        "Wq": rng.standard_normal((C, C), dtype=np.float32) * 0.05,
        "bq": rng.random(C, dtype=np.float32),
        "Wk": rng.standard_normal((C, C), dtype=np.float32) * 0.05,
        "bk": rng.random(C, dtype=np.float32),
        "Wv": rng.standard_normal((C, C), dtype=np.float32) * 0.05,
        "bv": rng.random(C, dtype=np.float32),
    }
    out = kernel(**inputs)
    print("kernel output:", out.shape, out.dtype, float(np.abs(out).max()))
